# revision 1
# baseline (speedup 1.0000x reference)
"""Symmetry-plane loss on 8 trn2 NeuronCores (Bass/Tile).

Shapes (hardcoded per spec):
  point_cloud    [64, 32768, 3] f32
  auxiliary_data [64, 32768, 3] f32   (closest-point grid, G = 32^3)
  voxel_data     [64, 32768, 1] f32   (occupancy)
  planes         [3, 64, 4]     f32
Returns scalar f32.

Sharding: pure data parallel, batch dim across the 8 cores (8 batches per
NeuronCore); host sums the 8 per-core scalar partials at the end.

Per-core layout/algorithm:
  - Q7 core j (partitions 16j..16j+15) owns batch j. Partitions 16j+{0,1,2,3}
    hold that batch's planar tables [1-vox, aux-eps (x,y,z)] (32768 f32 each)
    for the data-dependent voxel lookup via gpsimd ap_gather.
  - Points of each batch live across the 32 partitions of its quadrant
    (2 batches "A"/"B" per quadrant): point n = 1024*c + u sits at partition
    32q + c, free column u.
  - Voxel indices are rewrapped (%16 per Q7 core), gathered with ap_gather
    (idx position i = 32u + c), and the gather output rows are folded back to
    point-major layout with the DVE 32x32 stream transpose, which lands
    channel r of point (c, u) at (32q + c, 32u + r) - aligned with the points.
  - The Q7 SBUF port is shared with the DVE's second read port, so 2-input
    DVE ops stall the gathers (and vice versa).  The pipeline therefore leans
    on 1-input ops (tensor_scalar / copy / activation / reduce) and keeps the
    unavoidable 2-input ops (scalar_tensor_tensor chains) few and large.
"""
import os
import numpy as np

B, N, G, RES = 64, 32768, 32768, 32
NCORES = 8
NBL = 8            # batches per core
P = 3              # planes
WREG = 25.0
EPS = 1e-6
CH_U = 256         # u-chunk for index build
NIDX = int(os.environ.get("KBASS_NIDX", "4096"))  # idx per ap_gather
HALF = 2048        # transpose/pair half-chunk
NHALF = NIDX // HALF
UCH = HALF // 32   # u-chunk of pair phase (64)
NCH = N // NIDX    # gather chunks per plane
# voxel-axis quantization: y = 32*pts + 16 - (0.5 - 2^-13), then clamp and
# round-to-nearest(cast).  Equivalent to trunc+clip up to ~1e-4-wide bands
# at the cell boundaries (statistically ~0.01% of points).
SCALE_BIAS = 15.5001220703125
CLAMP_HI = 31.4375

_cache = {}


def _build_program():
    import concourse.bass as bass
    import concourse.tile as tile
    from concourse import bacc, mybir
    from contextlib import ExitStack

    f32 = mybir.dt.float32
    i16 = mybir.dt.int16
    Alu = mybir.AluOpType
    Act = mybir.ActivationFunctionType

    debug = bool(os.environ.get("KBASS_DEBUG"))
    skips = set(os.environ.get("KBASS_SKIP", "").split(","))
    repeat = int(os.environ.get("KBASS_REPEAT", "1"))

    nc = bacc.Bacc("TRN2", target_bir_lowering=False, debug=False)
    tab_d = nc.dram_tensor("tab", [NBL, 4, G], f32, kind="ExternalInput")
    pca_d = nc.dram_tensor("pca", [128, 3072], f32, kind="ExternalInput")
    pcb_d = nc.dram_tensor("pcb", [128, 3072], f32, kind="ExternalInput")
    pa_d = nc.dram_tensor("pa", [128, 12], f32, kind="ExternalInput")
    pb_d = nc.dram_tensor("pb", [128, 12], f32, kind="ExternalInput")
    bake_d = nc.dram_tensor("bake", [128, 2], f32, kind="ExternalInput")
    selm_d = nc.dram_tensor("selm", [128, 2], mybir.dt.int16, kind="ExternalInput")
    if debug:
        idx0_d = nc.dram_tensor("idx0", [128, 2048], i16, kind="ExternalOutput")
        tgo0_d = nc.dram_tensor("tgo0", [128, HALF], f32, kind="ExternalOutput")
        acc_d = nc.dram_tensor("accd", [128, P * NCH * NHALF], f32, kind="ExternalOutput")
    out_d = nc.dram_tensor("out", [1, 1], f32, kind="ExternalOutput")

    with tile.TileContext(nc) as tc, ExitStack() as ctx:
        cpool = ctx.enter_context(tc.tile_pool(name="const", bufs=1))
        gopool = ctx.enter_context(tc.tile_pool(name="go", bufs=1))
        tgopool = ctx.enter_context(tc.tile_pool(name="tgo", bufs=1))
        spool = ctx.enter_context(tc.tile_pool(name="scratch", bufs=1))
        psumpool = ctx.enter_context(tc.tile_pool(name="ps", bufs=1, space="PSUM"))

        TAB = cpool.tile([128, G], f32)
        for j in range(NBL):
            nc.sync.dma_start(TAB[16 * j:16 * j + 4, :], tab_d[j])
        PC = {}
        PC["A"] = cpool.tile([128, 3072], f32, tag="pca", name="pca_t")
        PC["B"] = cpool.tile([128, 3072], f32, tag="pcb", name="pcb_t")
        nc.sync.dma_start(PC["A"][:], pca_d[:])
        nc.sync.dma_start(PC["B"][:], pcb_d[:])
        PL = {}
        PL["A"] = cpool.tile([128, 12], f32, tag="pa", name="pa_t")
        PL["B"] = cpool.tile([128, 12], f32, tag="pb", name="pb_t")
        nc.sync.dma_start(PL["A"][:], pa_d[:])
        nc.sync.dma_start(PL["B"][:], pb_d[:])
        BAKE = cpool.tile([128, 2], f32, tag="bake")
        nc.sync.dma_start(BAKE[:], bake_d[:])
        SELM = cpool.tile([128, 2], i16, tag="selm")
        nc.sync.dma_start(SELM[:], selm_d[:])
        SELLO = SELM[:, 0:1]
        SELHI = SELM[:, 1:2]
        # one-time table bake: vox row -> (1 - v); aux rows -> t - eps
        nc.vector.tensor_scalar(TAB[:], TAB[:], BAKE[:, 0:1], BAKE[:, 1:2],
                                Alu.mult, Alu.add)

        # per-(plane,chunk) partial sums land here (one col per ACT sqrt)
        AACC = cpool.tile([128, P * NCH * NHALF], f32, tag="aacc")
        if "pair" in skips:
            nc.vector.memset(AACC[:], 0.0)

        # ---- per-(batch,plane) coefficients + reg term ----------------
        # PL row layout: [n0x n0y n0z d0 n1x n1y n1z d1 n2x n2y n2z d2]
        MN = {}    # [128, 9]: MN[:, 3p + c] = -2*n_pc / |n_p|^2
        M32 = {}   # [128, 9]: 32 * MN
        REG = {}   # [128, 1]
        CO = cpool.tile([128, 64], f32, tag="co")
        for xi, X in enumerate(("A", "B")):
            pl = PL[X]
            nxs = pl[:, 0:12:4]
            nys = pl[:, 1:12:4]
            nzs = pl[:, 2:12:4]
            base = xi * 32
            LN = CO[:, base:base + 3]
            T3 = CO[:, base + 3:base + 6]
            nc.vector.tensor_mul(LN, nxs, nxs)
            nc.vector.tensor_mul(T3, nys, nys)
            nc.vector.tensor_add(LN, LN, T3)
            nc.vector.tensor_mul(T3, nzs, nzs)
            nc.vector.tensor_add(LN, LN, T3)
            RL = CO[:, base + 6:base + 9]
            nc.vector.reciprocal(RL, LN)
            MN[X] = cpool.tile([128, 9], f32, tag="mn" + X, name="mn_" + X)
            for c, comp in enumerate((nxs, nys, nzs)):
                nc.vector.tensor_mul(MN[X][:, c:9:3], comp, RL)
            nc.vector.tensor_scalar_mul(MN[X][:], MN[X][:], -2.0)
            M32[X] = cpool.tile([128, 9], f32, tag="m32" + X, name="m32_" + X)
            nc.vector.tensor_scalar_mul(M32[X][:], MN[X][:], 32.0)
            # reg term
            SQ = CO[:, base + 9:base + 12]
            nc.scalar.activation(SQ, LN, Act.Sqrt)
            RS = CO[:, base + 12:base + 15]
            nc.vector.reciprocal(RS, SQ)
            NH = cpool.tile([128, 9], f32, tag="nh" + X)
            for c, comp in enumerate((nxs, nys, nzs)):
                nc.vector.tensor_mul(NH[:, 3 * c:3 * c + 3], comp, RS)
            NH3 = NH[:].rearrange("a (c p) -> a c p", c=3)
            NHT = NH[:].rearrange("a (c p) -> a p c", c=3)
            MT = cpool.tile([128, 9], f32, tag="mt" + X)
            MT3 = MT[:].rearrange("a (c p) -> a c p", c=3)
            nc.vector.tensor_tensor(MT3, NH3, NHT, Alu.mult)
            EYE = CO[:, base + 15:base + 24]
            nc.vector.memset(EYE, 0.0)
            for dpos in (15, 19, 23):
                nc.vector.memset(CO[:, base + dpos:base + dpos + 1], 1.0)
            nc.vector.tensor_sub(MT[:], MT[:], EYE)
            nc.vector.tensor_mul(MT[:], MT[:], MT[:])
            REG[X] = CO[:, base + 24:base + 25]
            nc.vector.tensor_reduce(REG[X], MT[:], mybir.AxisListType.X, Alu.add)

        GA = cpool.tile([128, 1024], i16, tag="ga")
        GB = cpool.tile([128, 1024], i16, tag="gb")
        GX = {"A": GA, "B": GB}
        IDX = cpool.tile([128, N // 16], i16, tag="idx")
        S = {}
        S["A"] = cpool.tile([128, 1024], f32, tag="sA", name="s_A")
        S["B"] = cpool.tile([128, 1024], f32, tag="sB", name="s_B")
        TAB3 = TAB[:].rearrange("a (n d) -> a n d", d=1)
        if "gphase" in skips:
            nc.vector.memset(GA[:], 0)
            nc.vector.memset(GB[:], 0)
        if "fixup" in skips:
            nc.vector.memset(IDX[:], 0)

        acol = 0
        for p in [pp for _ in range(repeat) for pp in range(P)]:
            for X in ([] if "gphase" in skips else ("A", "B")):
                pcr = PC[X][:].rearrange("a (u e) -> a u e", e=3)
                pl = PL[X]
                nx, ny, nz = (pl[:, 4 * p + c:4 * p + c + 1] for c in range(3))
                dd = pl[:, 4 * p + 3:4 * p + 4]
                # S = p . n + d  (full plane, 3 instrs)
                nc.vector.tensor_scalar(S[X][:], pcr[:, :, 0:1], nx, dd,
                                        Alu.mult, Alu.add)
                nc.vector.scalar_tensor_tensor(
                    S[X][:], pcr[:, :, 1:2], ny, S[X][:], Alu.mult, Alu.add)
                nc.vector.scalar_tensor_tensor(
                    S[X][:], pcr[:, :, 2:3], nz, S[X][:], Alu.mult, Alu.add)
                for k in range(1024 // CH_U):
                    u0 = k * CH_U
                    vi = []
                    for c in range(3):
                        px32 = spool.tile([128, CH_U], f32, tag=f"px{c}",
                                          name=f"px{c}_t")
                        nc.vector.tensor_scalar(px32[:],
                                                pcr[:, u0:u0 + CH_U, c:c + 1],
                                                32.0, SCALE_BIAS,
                                                Alu.mult, Alu.add)
                        nc.vector.scalar_tensor_tensor(
                            px32[:], S[X][:, u0:u0 + CH_U],
                            M32[X][:, 3 * p + c:3 * p + c + 1], px32[:],
                            Alu.mult, Alu.add)
                        nc.vector.tensor_scalar(px32[:], px32[:], 0.0, CLAMP_HI,
                                                Alu.max, Alu.min)
                        vc = spool.tile([128, CH_U], i16, tag=f"vi{c}",
                                        name=f"vi{c}_t")
                        nc.vector.tensor_copy(vc[:], px32[:])
                        vi.append(vc)
                    ti = spool.tile([128, CH_U], i16, tag="ti")
                    t2 = spool.tile([128, CH_U], i16, tag="t2")
                    nc.vector.tensor_scalar(ti[:], vi[0][:], 1024, None, Alu.mult)
                    nc.vector.tensor_scalar(t2[:], vi[1][:], 32, None, Alu.mult)
                    nc.vector.tensor_add(ti[:], ti[:], t2[:])
                    nc.vector.tensor_tensor(GX[X][:, u0:u0 + CH_U], ti[:],
                                            vi[2][:], Alu.add)
            # ---- rewrap %16 per core (shuffle + masked overwrite) --
            if "fixup" not in skips:
                swap = list(range(16, 32)) + list(range(16))
                T1 = gopool.tile([128, 1024], i16, tag="go", name="t1s_t")
                nc.vector.stream_shuffle(T1[:], GA[:], swap)
                nc.vector.tensor_copy(IDX[:, 1:2048:2], GB[:])
                nc.vector.copy_predicated(IDX[:, 1:2048:2],
                                          SELLO[:].to_broadcast([128, 1024]), T1[:])
                T2 = gopool.tile([128, 1024], i16, tag="go", name="t2s_t")
                nc.vector.stream_shuffle(T2[:], GB[:], swap)
                nc.vector.tensor_copy(IDX[:, 0:2048:2], GA[:])
                nc.vector.copy_predicated(IDX[:, 0:2048:2],
                                          SELHI[:].to_broadcast([128, 1024]), T2[:])
            if debug and p == 0:
                nc.sync.dma_start(idx0_d[:], IDX[:])
            # ---- gather + pair ------------------------------------
            for k in range(NCH):
                GO = gopool.tile([128, NIDX], f32, tag="go")
                if "gather" not in skips:
                    nc.gpsimd.ap_gather(
                        GO[:], TAB3, IDX[:, (NIDX // 16) * k:(NIDX // 16) * (k + 1)],
                        channels=128, num_elems=G, d=1, num_idxs=NIDX)
                else:
                    nc.gpsimd.ap_gather(GO[:, 0:4], TAB3, IDX[:, 0:1],
                                        channels=128, num_elems=G, d=1, num_idxs=4)
                    nc.vector.memset(GO[:, 4:NIDX], 0)
                for h in range(NHALF):
                    TGO = tgopool.tile([128, HALF], f32, tag="tgo")
                    if "transpose" not in skips:
                        nc.vector.transpose(TGO[:], GO[:, HALF * h:HALF * (h + 1)])
                    if debug and p == 0 and k == 0 and h == 0:
                        nc.sync.dma_start(tgo0_d[:], TGO[:])
                    tgor = TGO[:].rearrange("a (v r) -> a v r", r=32)
                    u0 = UCH * (NHALF * k + h)
                    if "pair" in skips:
                        continue
                    # reflected pts for both halves -> OAB[:, u, xi, c]
                    OAB = spool.tile([128, UCH * 6], f32, tag="oab")
                    oabr = OAB[:].rearrange("a (u x c) -> a u x c", x=2, c=3)
                    for xi, X in enumerate(("A", "B")):
                        pcr = PC[X][:].rearrange("a (u e) -> a u e", e=3)
                        for c in range(3):
                            nc.vector.scalar_tensor_tensor(
                                oabr[:, :, xi, c:c + 1],
                                S[X][:, u0:u0 + UCH],
                                MN[X][:, 3 * p + c:3 * p + c + 1],
                                pcr[:, u0:u0 + UCH, c:c + 1],
                                Alu.mult, Alu.add)
                    # dx = o - t' (t' rows 1..3 of each half, pre-baked t-eps)
                    DX = spool.tile([128, UCH * 6], f32, tag="dx")
                    dxr = DX[:].rearrange("a (u x c) -> a u x c", x=2, c=3)
                    nc.vector.scalar_tensor_tensor(
                        dxr[:, :, 0, :], tgor[:, :, 1:4], -1.0, oabr[:, :, 0, :],
                        Alu.mult, Alu.add)
                    nc.vector.scalar_tensor_tensor(
                        dxr[:, :, 1, :], tgor[:, :, 17:20], -1.0, oabr[:, :, 1, :],
                        Alu.mult, Alu.add)
                    SQ = spool.tile([128, UCH * 6], f32, tag="sq")
                    nc.scalar.activation(SQ[:], DX[:], Act.Square)
                    D2 = spool.tile([128, UCH * 2], f32, tag="d2")
                    d2r = D2[:].rearrange("a (u x) -> a u x", x=2)
                    nc.vector.tensor_reduce(
                        d2r, SQ[:].rearrange("a (u x c) -> a u x c", x=2, c=3),
                        mybir.AxisListType.X, Alu.add)
                    # weight by (1 - v)^2 under the sqrt; w pre-baked in row 0
                    wsl = tgor[:, :, 0:17:16]          # [128, UCH, 2]
                    nc.vector.tensor_tensor(d2r, d2r, wsl, Alu.mult)
                    nc.vector.tensor_tensor(d2r, d2r, wsl, Alu.mult)
                    DIST = spool.tile([128, UCH * 2], f32, tag="dist")
                    nc.scalar.activation(DIST[:], D2[:], Act.Sqrt,
                                         accum_out=AACC[:, acol:acol + 1])
                    acol += 1
            acol = acol % (P * NCH * NHALF)

        # ---- final reduction ----------------------------------------
        if debug:
            nc.sync.dma_start(acc_d[:], AACC[:])
        RED = cpool.tile([128, 3], f32, tag="red")
        nc.vector.tensor_reduce(RED[:, 0:1], AACC[:], mybir.AxisListType.X, Alu.add)
        nc.vector.tensor_copy(RED[:, 1:2], REG["A"])
        nc.vector.tensor_copy(RED[:, 2:3], REG["B"])
        ONES = cpool.tile([128, 1], f32, tag="ones")
        nc.vector.memset(ONES[:], 1.0)
        PS = psumpool.tile([1, 3], f32)
        nc.tensor.matmul(out=PS[:], lhsT=ONES[:], rhs=RED[:], start=True, stop=True)
        SC = cpool.tile([1, 3], f32, tag="sc")
        nc.vector.tensor_copy(SC[:], PS[:])
        F = cpool.tile([1, 2], f32, tag="f")
        nc.vector.tensor_add(F[:, 0:1], SC[:, 1:2], SC[:, 2:3])
        nc.vector.tensor_scalar(F[:, 0:1], F[:, 0:1], WREG / (32.0 * B), None,
                                Alu.mult)
        nc.vector.tensor_scalar(F[:, 1:2], SC[:, 0:1], 1.0 / B, None, Alu.mult)
        OUT = cpool.tile([1, 1], f32, tag="out")
        nc.vector.tensor_add(OUT[:], F[:, 0:1], F[:, 1:2])
        nc.sync.dma_start(out_d[:], OUT[:])

    nc.compile()
    return nc


def _make_callable(nc, n_cores=NCORES):
    import jax
    import numpy as np
    from jax.sharding import Mesh, PartitionSpec
    from jax.experimental.shard_map import shard_map
    from concourse import mybir, bass2jax
    from concourse.bass2jax import _bass_exec_p, install_neuronx_cc_hook

    install_neuronx_cc_hook()
    partition_name = nc.partition_id_tensor.name if nc.partition_id_tensor else None
    in_names, out_names, out_avals, zero_outs = [], [], [], []
    for alloc in nc.m.functions[0].allocations:
        if not isinstance(alloc, mybir.MemoryLocationSet):
            continue
        name = alloc.memorylocations[0].name
        if alloc.kind == "ExternalInput":
            if name != partition_name:
                in_names.append(name)
        elif alloc.kind == "ExternalOutput":
            out_names.append(name)
            shape = tuple(alloc.tensor_shape)
            dtype = mybir.dt.np(alloc.dtype)
            out_avals.append(jax.core.ShapedArray(shape, dtype))
            zero_outs.append(np.zeros(shape, dtype))
    n_params = len(in_names)
    all_in_names = list(in_names) + list(out_names)
    if partition_name is not None:
        all_in_names.append(partition_name)

    def _body(*args):
        operands = list(args)
        if partition_name is not None:
            operands.append(bass2jax.partition_id_tensor())
        outs = _bass_exec_p.bind(
            *operands,
            out_avals=tuple(out_avals),
            in_names=tuple(all_in_names),
            out_names=tuple(out_names),
            lowering_input_output_aliases=(),
            sim_require_finite=True,
            sim_require_nnan=True,
            nc=nc,
        )
        return tuple(outs)

    devices = jax.devices()[:n_cores]
    mesh = Mesh(np.asarray(devices), ("core",))
    n_outs = len(out_avals)
    sharded = jax.jit(
        shard_map(_body, mesh=mesh,
                  in_specs=(PartitionSpec("core"),) * (n_params + n_outs),
                  out_specs=(PartitionSpec("core"),) * n_outs,
                  check_rep=False),
        keep_unused=True,
    )
    return sharded, in_names, out_names, out_avals, zero_outs


def _get_exec():
    if "exec" not in _cache:
        nc = _build_program()
        _cache["exec"] = _make_callable(nc)
    return _cache["exec"]


def _shard_inputs(pc, aux, vox, planes):
    """Layout-only host prep: per-core input dict list."""
    planes_b = np.ascontiguousarray(planes.transpose(1, 0, 2)).reshape(B, 12)
    bake = np.ones((128, 2), np.float32)
    bake[:, 1] = -EPS
    bake[0::16, 0] = -1.0
    bake[0::16, 1] = 1.0
    selm = np.zeros((128, 2), np.int16)
    selm[(np.arange(128) % 32) < 16, 0] = 1
    selm[(np.arange(128) % 32) >= 16, 1] = 1
    in_maps = []
    for core in range(NCORES):
        sl = slice(NBL * core, NBL * (core + 1))
        tab = np.empty((NBL, 4, G), np.float32)
        tab[:, 0, :] = vox[sl, :, 0]
        tab[:, 1:4, :] = aux[sl].transpose(0, 2, 1)
        pcs = pc[sl]                                   # [8, N, 3]
        pca = np.ascontiguousarray(pcs[0::2]).reshape(128, 3072)
        pcb = np.ascontiguousarray(pcs[1::2]).reshape(128, 3072)
        pa = np.repeat(planes_b[sl][0::2], 32, axis=0)
        pb = np.repeat(planes_b[sl][1::2], 32, axis=0)
        in_maps.append({"tab": tab, "pca": pca, "pcb": pcb,
                        "pa": np.ascontiguousarray(pa),
                        "pb": np.ascontiguousarray(pb),
                        "bake": bake, "selm": selm})
    return in_maps


def kernel(point_cloud, auxiliary_data, voxel_data, planes):
    import jax
    pc = np.asarray(point_cloud, dtype=np.float32)
    aux = np.asarray(auxiliary_data, dtype=np.float32)
    vox = np.asarray(voxel_data, dtype=np.float32)
    pl = np.asarray(planes, dtype=np.float32)

    sharded, in_names, out_names, out_avals, zero_outs = _get_exec()
    in_maps = _shard_inputs(pc, aux, vox, pl)
    concat_in = [np.concatenate([m[name] for m in in_maps], axis=0)
                 for name in in_names]
    concat_zero = [np.zeros((NCORES * z.shape[0], *z.shape[1:]), z.dtype)
                   for z in zero_outs]
    outs = sharded(*[jax.device_put(a) for a in concat_in + concat_zero])
    jax.block_until_ready(outs)
    o = np.asarray(outs[out_names.index("out")]).reshape(NCORES)
    return np.float32(o.sum(dtype=np.float64))



# revision 11
# speedup vs baseline: 1.0059x; 1.0059x over previous
"""Symmetry-plane loss on 8 trn2 NeuronCores (Bass/Tile).

Shapes (hardcoded per spec):
  point_cloud    [64, 32768, 3] f32
  auxiliary_data [64, 32768, 3] f32   (closest-point grid, G = 32^3)
  voxel_data     [64, 32768, 1] f32   (occupancy)
  planes         [3, 64, 4]     f32
Returns scalar f32.

Sharding: pure data parallel, batch dim across the 8 cores (8 batches per
NeuronCore); host sums the 8 per-core scalar partials at the end.

Per-core layout/algorithm:
  - Q7 core j (partitions 16j..16j+15) owns batch j. Partitions 16j+{0,1,2,3}
    hold that batch's planar tables [1-vox, aux-eps (x,y,z)] (32768 f32 each)
    for the data-dependent voxel lookup via gpsimd ap_gather.
  - Points of each batch live across the 32 partitions of its quadrant
    (2 batches "A"/"B" per quadrant): point n = 1024*c + u sits at partition
    32q + c, free column u.
  - Voxel indices are rewrapped (%16 per Q7 core), gathered with ap_gather
    (idx position i = 32u + c), and the gather output rows are folded back to
    point-major layout with the DVE 32x32 stream transpose, which lands
    channel r of point (c, u) at (32q + c, 32u + r) - aligned with the points.
  - The Q7 SBUF port is shared with the DVE's second read port, so 2-input
    DVE ops stall the gathers (and vice versa).  The pipeline therefore leans
    on 1-input ops (tensor_scalar / copy / activation / reduce) and keeps the
    unavoidable 2-input ops (scalar_tensor_tensor chains) few and large.
"""
import os
import numpy as np

B, N, G, RES = 64, 32768, 32768, 32
NCORES = 8
NBL = 8            # batches per core
P = 3              # planes
WREG = 25.0
EPS = 1e-6
CH_U = 256         # u-chunk for index build
NIDX = int(os.environ.get("KBASS_NIDX", "2048"))  # idx per gather chunk
GATHER = os.environ.get("KBASS_GATHER", "ic")  # ic=pool indirect_copy, ap=gpsimd ap_gather
GOBUFS = int(os.environ.get("KBASS_GOBUFS", "2"))
TGOBUFS = int(os.environ.get("KBASS_TGOBUFS", "1"))
HALF = 2048        # transpose/pair half-chunk
NHALF = NIDX // HALF
UCH = HALF // 32   # u-chunk of pair phase (64)
NCH = N // NIDX    # gather chunks per plane
# voxel-axis quantization: y = 32*pts + 16 - (0.5 - 2^-13), then clamp and
# round-to-nearest(cast).  Equivalent to trunc+clip up to ~1e-4-wide bands
# at the cell boundaries (statistically ~0.01% of points).
SCALE_BIAS = 15.5001220703125
CLAMP_HI = 31.4375

_cache = {}


def _build_program():
    import concourse.bass as bass
    import concourse.tile as tile
    from concourse import bacc, mybir
    from contextlib import ExitStack

    f32 = mybir.dt.float32
    i16 = mybir.dt.uint16 if GATHER == "ic" else mybir.dt.int16
    Alu = mybir.AluOpType
    Act = mybir.ActivationFunctionType

    debug = bool(os.environ.get("KBASS_DEBUG"))
    skips = set(os.environ.get("KBASS_SKIP", "").split(","))
    repeat = int(os.environ.get("KBASS_REPEAT", "1"))

    nc = bacc.Bacc("TRN2", target_bir_lowering=False, debug=False)
    tab_d = nc.dram_tensor("tab", [NBL, 4, G], f32, kind="ExternalInput")
    pca_d = nc.dram_tensor("pca", [128, 3072], f32, kind="ExternalInput")
    pcb_d = nc.dram_tensor("pcb", [128, 3072], f32, kind="ExternalInput")
    pa_d = nc.dram_tensor("pa", [128, 12], f32, kind="ExternalInput")
    pb_d = nc.dram_tensor("pb", [128, 12], f32, kind="ExternalInput")
    bake_d = nc.dram_tensor("bake", [128, 2], f32, kind="ExternalInput")
    selm_d = nc.dram_tensor("selm", [128, 2], i16, kind="ExternalInput")
    if debug:
        idx0_d = nc.dram_tensor("idx0", [128, 2048], i16, kind="ExternalOutput")
        tgo0_d = nc.dram_tensor("tgo0", [128, HALF], f32, kind="ExternalOutput")
        acc_d = nc.dram_tensor("accd", [128, P * NCH * NHALF], f32, kind="ExternalOutput")
    out_d = nc.dram_tensor("out", [1, 1], f32, kind="ExternalOutput")

    with tile.TileContext(nc) as tc, ExitStack() as ctx:
        cpool = ctx.enter_context(tc.tile_pool(name="const", bufs=1))
        gopool = ctx.enter_context(tc.tile_pool(name="go", bufs=GOBUFS))
        tgopool = ctx.enter_context(tc.tile_pool(name="tgo", bufs=TGOBUFS))
        spool = ctx.enter_context(tc.tile_pool(name="scratch", bufs=1))
        psumpool = ctx.enter_context(tc.tile_pool(name="ps", bufs=1, space="PSUM"))

        TAB = cpool.tile([128, G], f32)
        for j in range(NBL):
            nc.sync.dma_start(TAB[16 * j:16 * j + 4, :], tab_d[j])
        PC = {}
        PC["A"] = cpool.tile([128, 3072], f32, tag="pca", name="pca_t")
        PC["B"] = cpool.tile([128, 3072], f32, tag="pcb", name="pcb_t")
        nc.sync.dma_start(PC["A"][:], pca_d[:])
        nc.sync.dma_start(PC["B"][:], pcb_d[:])
        PL = {}
        PL["A"] = cpool.tile([128, 12], f32, tag="pa", name="pa_t")
        PL["B"] = cpool.tile([128, 12], f32, tag="pb", name="pb_t")
        nc.sync.dma_start(PL["A"][:], pa_d[:])
        nc.sync.dma_start(PL["B"][:], pb_d[:])
        BAKE = cpool.tile([128, 2], f32, tag="bake")
        nc.sync.dma_start(BAKE[:], bake_d[:])
        SELM = cpool.tile([128, 2], i16, tag="selm")
        nc.sync.dma_start(SELM[:], selm_d[:])
        SELLO = SELM[:, 0:1]
        SELHI = SELM[:, 1:2]
        # one-time table bake: vox row -> (1 - v); aux rows -> t - eps
        # (on ACT engine to keep DVE free)
        nc.scalar.activation(TAB[:], TAB[:], Act.Identity,
                             bias=BAKE[:, 1:2], scale=BAKE[:, 0:1])

        # per-(plane,chunk) partial sums land here (one col per ACT sqrt)
        AACC = cpool.tile([128, P * NCH * NHALF], f32, tag="aacc")
        if "pair" in skips:
            nc.vector.memset(AACC[:], 0.0)

        # ---- per-(batch,plane) coefficients + reg term ----------------
        # PL row layout: [n0x n0y n0z d0 n1x n1y n1z d1 n2x n2y n2z d2]
        MN = {}    # [128, 9]: MN[:, 3p + c] = -2*n_pc / |n_p|^2
        M32 = {}   # [128, 9]: 32 * MN
        REG = {}   # [128, 1]
        CO = cpool.tile([128, 64], f32, tag="co")
        for xi, X in enumerate(("A", "B")):
            pl = PL[X]
            nxs = pl[:, 0:12:4]
            nys = pl[:, 1:12:4]
            nzs = pl[:, 2:12:4]
            base = xi * 32
            LN = CO[:, base:base + 3]
            T3 = CO[:, base + 3:base + 6]
            nc.vector.tensor_mul(LN, nxs, nxs)
            nc.vector.tensor_mul(T3, nys, nys)
            nc.vector.tensor_add(LN, LN, T3)
            nc.vector.tensor_mul(T3, nzs, nzs)
            nc.vector.tensor_add(LN, LN, T3)
            RL = CO[:, base + 6:base + 9]
            nc.vector.reciprocal(RL, LN)
            MN[X] = cpool.tile([128, 9], f32, tag="mn" + X, name="mn_" + X)
            for c, comp in enumerate((nxs, nys, nzs)):
                nc.vector.tensor_mul(MN[X][:, c:9:3], comp, RL)
            nc.vector.tensor_scalar_mul(MN[X][:], MN[X][:], -2.0)
            M32[X] = cpool.tile([128, 9], f32, tag="m32" + X, name="m32_" + X)
            nc.vector.tensor_scalar_mul(M32[X][:], MN[X][:], 32.0)
            # reg term
            SQ = CO[:, base + 9:base + 12]
            nc.scalar.activation(SQ, LN, Act.Sqrt)
            RS = CO[:, base + 12:base + 15]
            nc.vector.reciprocal(RS, SQ)
            NH = cpool.tile([128, 9], f32, tag="nh" + X)
            for c, comp in enumerate((nxs, nys, nzs)):
                nc.vector.tensor_mul(NH[:, 3 * c:3 * c + 3], comp, RS)
            NH3 = NH[:].rearrange("a (c p) -> a c p", c=3)
            NHT = NH[:].rearrange("a (c p) -> a p c", c=3)
            MT = cpool.tile([128, 9], f32, tag="mt" + X)
            MT3 = MT[:].rearrange("a (c p) -> a c p", c=3)
            nc.vector.tensor_tensor(MT3, NH3, NHT, Alu.mult)
            EYE = CO[:, base + 15:base + 24]
            nc.vector.memset(EYE, 0.0)
            for dpos in (15, 19, 23):
                nc.vector.memset(CO[:, base + dpos:base + dpos + 1], 1.0)
            nc.vector.tensor_sub(MT[:], MT[:], EYE)
            nc.vector.tensor_mul(MT[:], MT[:], MT[:])
            REG[X] = CO[:, base + 24:base + 25]
            nc.vector.tensor_reduce(REG[X], MT[:], mybir.AxisListType.X, Alu.add)

        GA = cpool.tile([128, 1024], i16, tag="ga")
        GB = cpool.tile([128, 1024], i16, tag="gb")
        GX = {"A": GA, "B": GB}
        IDX = cpool.tile([128, N // 16], i16, tag="idx")
        S = {}
        S["A"] = cpool.tile([128, 1024], f32, tag="sA", name="s_A")
        S["B"] = cpool.tile([128, 1024], f32, tag="sB", name="s_B")
        TAB3 = TAB[:].rearrange("a (n d) -> a n d", d=1)
        if "gphase" in skips:
            nc.vector.memset(GA[:], 0)
            nc.vector.memset(GB[:], 0)
        if "fixup" in skips:
            nc.vector.memset(IDX[:], 0)

        acol = 0
        for p in [pp for _ in range(repeat) for pp in range(P)]:
            for X in ([] if "gphase" in skips else ("A", "B")):
                pcr = PC[X][:].rearrange("a (u e) -> a u e", e=3)
                pl = PL[X]
                nx, ny, nz = (pl[:, 4 * p + c:4 * p + c + 1] for c in range(3))
                dd = pl[:, 4 * p + 3:4 * p + 4]
                # S = p . n + d  (full plane, 3 instrs)
                nc.vector.tensor_scalar(S[X][:], pcr[:, :, 0:1], nx, dd,
                                        Alu.mult, Alu.add)
                nc.vector.scalar_tensor_tensor(
                    S[X][:], pcr[:, :, 1:2], ny, S[X][:], Alu.mult, Alu.add)
                nc.vector.scalar_tensor_tensor(
                    S[X][:], pcr[:, :, 2:3], nz, S[X][:], Alu.mult, Alu.add)
                for k in range(1024 // CH_U):
                    u0 = k * CH_U
                    vi = []
                    for c in range(3):
                        px32 = spool.tile([128, CH_U], f32, tag=f"px{c}",
                                          name=f"px{c}_t")
                        nc.vector.tensor_scalar(px32[:],
                                                pcr[:, u0:u0 + CH_U, c:c + 1],
                                                32.0, SCALE_BIAS,
                                                Alu.mult, Alu.add)
                        nc.vector.scalar_tensor_tensor(
                            px32[:], S[X][:, u0:u0 + CH_U],
                            M32[X][:, 3 * p + c:3 * p + c + 1], px32[:],
                            Alu.mult, Alu.add)
                        nc.vector.tensor_scalar(px32[:], px32[:], 0.0, CLAMP_HI,
                                                Alu.max, Alu.min)
                        vc = spool.tile([128, CH_U], i16, tag=f"vi{c}",
                                        name=f"vi{c}_t")
                        nc.vector.tensor_copy(vc[:], px32[:])
                        vi.append(vc)
                    ti = spool.tile([128, CH_U], i16, tag="ti")
                    t2 = spool.tile([128, CH_U], i16, tag="t2")
                    nc.vector.tensor_scalar(ti[:], vi[0][:], 1024, None, Alu.mult)
                    nc.vector.tensor_scalar(t2[:], vi[1][:], 32, None, Alu.mult)
                    nc.vector.tensor_add(ti[:], ti[:], t2[:])
                    nc.vector.tensor_tensor(GX[X][:, u0:u0 + CH_U], ti[:],
                                            vi[2][:], Alu.add)
            # ---- rewrap %16 per core (shuffle + masked overwrite) --
            if "fixup" not in skips:
                swap = list(range(16, 32)) + list(range(16))
                T1 = gopool.tile([128, 1024], i16, tag="go", name="t1s_t")
                nc.vector.stream_shuffle(T1[:], GA[:], swap)
                nc.vector.tensor_copy(IDX[:, 1:2048:2], GB[:])
                nc.vector.copy_predicated(IDX[:, 1:2048:2],
                                          SELLO[:].to_broadcast([128, 1024]), T1[:])
                T2 = gopool.tile([128, 1024], i16, tag="go", name="t2s_t")
                nc.vector.stream_shuffle(T2[:], GB[:], swap)
                nc.vector.tensor_copy(IDX[:, 0:2048:2], GA[:])
                nc.vector.copy_predicated(IDX[:, 0:2048:2],
                                          SELHI[:].to_broadcast([128, 1024]), T2[:])
            if debug and p == 0:
                nc.sync.dma_start(idx0_d[:], IDX[:])
            # ---- gather + pair ------------------------------------
            for k in range(NCH):
                GO = gopool.tile([128, NIDX], f32, tag="go")
                idx_sl = IDX[:, (NIDX // 16) * k:(NIDX // 16) * (k + 1)]
                if "gather" not in skips:
                    if GATHER == "ic":
                        # ISA: IndirectCopy dst elem count <= 1024 per inst
                        for s in range(NIDX // 1024):
                            nc.gpsimd.indirect_copy(
                                GO[:, 1024 * s:1024 * (s + 1)], TAB[:],
                                IDX[:, (NIDX // 16) * k + 64 * s:
                                    (NIDX // 16) * k + 64 * (s + 1)],
                                i_know_ap_gather_is_preferred=True)
                    else:
                        nc.gpsimd.ap_gather(
                            GO[:], TAB3, idx_sl,
                            channels=128, num_elems=G, d=1, num_idxs=NIDX)
                else:
                    nc.gpsimd.ap_gather(GO[:, 0:4], TAB3, IDX[:, 0:1],
                                        channels=128, num_elems=G, d=1, num_idxs=4)
                    nc.vector.memset(GO[:, 4:NIDX], 0)
                for h in range(NHALF):
                    TGO = tgopool.tile([128, HALF], f32, tag="tgo")
                    if "transpose" not in skips:
                        nc.vector.transpose(TGO[:], GO[:, HALF * h:HALF * (h + 1)])
                    if debug and p == 0 and k == 0 and h == 0:
                        nc.sync.dma_start(tgo0_d[:], TGO[:])
                    tgor = TGO[:].rearrange("a (v r) -> a v r", r=32)
                    u0 = UCH * (NHALF * k + h)
                    if "pair" in skips:
                        continue
                    # reflected pts for both halves -> OAB[:, u, xi, c]
                    OAB = spool.tile([128, UCH * 6], f32, tag="oab")
                    oabr = OAB[:].rearrange("a (u x c) -> a u x c", x=2, c=3)
                    for xi, X in enumerate(("A", "B")):
                        pcr = PC[X][:].rearrange("a (u e) -> a u e", e=3)
                        for c in range(3):
                            nc.vector.scalar_tensor_tensor(
                                oabr[:, :, xi, c:c + 1],
                                S[X][:, u0:u0 + UCH],
                                MN[X][:, 3 * p + c:3 * p + c + 1],
                                pcr[:, u0:u0 + UCH, c:c + 1],
                                Alu.mult, Alu.add)
                    # dx = o - t' (t' rows 1..3 of each half, pre-baked t-eps)
                    DX = spool.tile([128, UCH * 6], f32, tag="dx")
                    dxr = DX[:].rearrange("a (u x c) -> a u x c", x=2, c=3)
                    nc.vector.scalar_tensor_tensor(
                        dxr[:, :, 0, :], tgor[:, :, 1:4], -1.0, oabr[:, :, 0, :],
                        Alu.mult, Alu.add)
                    nc.vector.scalar_tensor_tensor(
                        dxr[:, :, 1, :], tgor[:, :, 17:20], -1.0, oabr[:, :, 1, :],
                        Alu.mult, Alu.add)
                    SQ = spool.tile([128, UCH * 6], f32, tag="sq")
                    nc.scalar.activation(SQ[:], DX[:], Act.Square)
                    D2 = spool.tile([128, UCH * 2], f32, tag="d2")
                    d2r = D2[:].rearrange("a (u x) -> a u x", x=2)
                    nc.vector.tensor_reduce(
                        d2r, SQ[:].rearrange("a (u x c) -> a u x c", x=2, c=3),
                        mybir.AxisListType.X, Alu.add)
                    # weight by (1 - v)^2 under the sqrt; w pre-baked in row 0
                    wsl = tgor[:, :, 0:17:16]          # [128, UCH, 2]
                    nc.vector.tensor_tensor(d2r, d2r, wsl, Alu.mult)
                    nc.vector.tensor_tensor(d2r, d2r, wsl, Alu.mult)
                    DIST = spool.tile([128, UCH * 2], f32, tag="dist")
                    nc.scalar.activation(DIST[:], D2[:], Act.Sqrt,
                                         accum_out=AACC[:, acol:acol + 1])
                    acol += 1
            acol = acol % (P * NCH * NHALF)

        # ---- final reduction ----------------------------------------
        if debug:
            nc.sync.dma_start(acc_d[:], AACC[:])
        RED = cpool.tile([128, 3], f32, tag="red")
        nc.vector.tensor_reduce(RED[:, 0:1], AACC[:], mybir.AxisListType.X, Alu.add)
        nc.vector.tensor_copy(RED[:, 1:2], REG["A"])
        nc.vector.tensor_copy(RED[:, 2:3], REG["B"])
        ONES = cpool.tile([128, 1], f32, tag="ones")
        nc.vector.memset(ONES[:], 1.0)
        PS = psumpool.tile([1, 3], f32)
        nc.tensor.matmul(out=PS[:], lhsT=ONES[:], rhs=RED[:], start=True, stop=True)
        SC = cpool.tile([1, 3], f32, tag="sc")
        nc.vector.tensor_copy(SC[:], PS[:])
        F = cpool.tile([1, 2], f32, tag="f")
        nc.vector.tensor_add(F[:, 0:1], SC[:, 1:2], SC[:, 2:3])
        nc.vector.tensor_scalar(F[:, 0:1], F[:, 0:1], WREG / (32.0 * B), None,
                                Alu.mult)
        nc.vector.tensor_scalar(F[:, 1:2], SC[:, 0:1], 1.0 / B, None, Alu.mult)
        OUT = cpool.tile([1, 1], f32, tag="out")
        nc.vector.tensor_add(OUT[:], F[:, 0:1], F[:, 1:2])
        nc.sync.dma_start(out_d[:], OUT[:])

    nc.compile()
    return nc


def _make_callable(nc, n_cores=NCORES):
    import jax
    import numpy as np
    from jax.sharding import Mesh, PartitionSpec
    from jax.experimental.shard_map import shard_map
    from concourse import mybir, bass2jax
    from concourse.bass2jax import _bass_exec_p, install_neuronx_cc_hook

    install_neuronx_cc_hook()
    partition_name = nc.partition_id_tensor.name if nc.partition_id_tensor else None
    in_names, out_names, out_avals, zero_outs = [], [], [], []
    for alloc in nc.m.functions[0].allocations:
        if not isinstance(alloc, mybir.MemoryLocationSet):
            continue
        name = alloc.memorylocations[0].name
        if alloc.kind == "ExternalInput":
            if name != partition_name:
                in_names.append(name)
        elif alloc.kind == "ExternalOutput":
            out_names.append(name)
            shape = tuple(alloc.tensor_shape)
            dtype = mybir.dt.np(alloc.dtype)
            out_avals.append(jax.core.ShapedArray(shape, dtype))
            zero_outs.append(np.zeros(shape, dtype))
    n_params = len(in_names)
    all_in_names = list(in_names) + list(out_names)
    if partition_name is not None:
        all_in_names.append(partition_name)

    def _body(*args):
        operands = list(args)
        if partition_name is not None:
            operands.append(bass2jax.partition_id_tensor())
        outs = _bass_exec_p.bind(
            *operands,
            out_avals=tuple(out_avals),
            in_names=tuple(all_in_names),
            out_names=tuple(out_names),
            lowering_input_output_aliases=(),
            sim_require_finite=True,
            sim_require_nnan=True,
            nc=nc,
        )
        return tuple(outs)

    devices = jax.devices()[:n_cores]
    mesh = Mesh(np.asarray(devices), ("core",))
    n_outs = len(out_avals)
    sharded = jax.jit(
        shard_map(_body, mesh=mesh,
                  in_specs=(PartitionSpec("core"),) * (n_params + n_outs),
                  out_specs=(PartitionSpec("core"),) * n_outs,
                  check_rep=False),
        keep_unused=True,
    )
    return sharded, in_names, out_names, out_avals, zero_outs


def _get_exec():
    if "exec" not in _cache:
        nc = _build_program()
        _cache["exec"] = _make_callable(nc)
    return _cache["exec"]


def _shard_inputs(pc, aux, vox, planes):
    """Layout-only host prep: per-core input dict list."""
    planes_b = np.ascontiguousarray(planes.transpose(1, 0, 2)).reshape(B, 12)
    bake = np.ones((128, 2), np.float32)
    bake[:, 1] = -EPS
    bake[0::16, 0] = -1.0
    bake[0::16, 1] = 1.0
    selm_dt = np.uint16 if GATHER == "ic" else np.int16
    selm = np.zeros((128, 2), selm_dt)
    selm[(np.arange(128) % 32) < 16, 0] = 1
    selm[(np.arange(128) % 32) >= 16, 1] = 1
    in_maps = []
    for core in range(NCORES):
        sl = slice(NBL * core, NBL * (core + 1))
        tab = np.empty((NBL, 4, G), np.float32)
        tab[:, 0, :] = vox[sl, :, 0]
        tab[:, 1:4, :] = aux[sl].transpose(0, 2, 1)
        pcs = pc[sl]                                   # [8, N, 3]
        pca = np.ascontiguousarray(pcs[0::2]).reshape(128, 3072)
        pcb = np.ascontiguousarray(pcs[1::2]).reshape(128, 3072)
        pa = np.repeat(planes_b[sl][0::2], 32, axis=0)
        pb = np.repeat(planes_b[sl][1::2], 32, axis=0)
        in_maps.append({"tab": tab, "pca": pca, "pcb": pcb,
                        "pa": np.ascontiguousarray(pa),
                        "pb": np.ascontiguousarray(pb),
                        "bake": bake, "selm": selm})
    return in_maps


def kernel(point_cloud, auxiliary_data, voxel_data, planes):
    import jax
    pc = np.asarray(point_cloud, dtype=np.float32)
    aux = np.asarray(auxiliary_data, dtype=np.float32)
    vox = np.asarray(voxel_data, dtype=np.float32)
    pl = np.asarray(planes, dtype=np.float32)

    sharded, in_names, out_names, out_avals, zero_outs = _get_exec()
    in_maps = _shard_inputs(pc, aux, vox, pl)
    concat_in = [np.concatenate([m[name] for m in in_maps], axis=0)
                 for name in in_names]
    concat_zero = [np.zeros((NCORES * z.shape[0], *z.shape[1:]), z.dtype)
                   for z in zero_outs]
    outs = sharded(*[jax.device_put(a) for a in concat_in + concat_zero])
    jax.block_until_ready(outs)
    o = np.asarray(outs[out_names.index("out")]).reshape(NCORES)
    return np.float32(o.sum(dtype=np.float64))

